# revision 30
# baseline (speedup 1.0000x reference)
"""Multi-head self-attention (B=4, S=2048, hidden=1024, 16 heads, d_k=64,
causal) on 8 Trainium2 NeuronCores.

Sharding: core c handles batch b = c//2 and head-group hg = c%2 (8 heads =
512 hidden dims). Each core computes Q/K/V for its heads, causal attention,
and a partial output projection against its wo column-slice; the host sums
the two partials per batch and adds the (bo + wo@bv) bias.

v3 design (fp16, pair-tiled, software-pipelined):
  - All matmul operands are fp16: 1 cycle/row on the PE (vs the fp32
    HIGH-power mode's ~2-3), FWL fast weight loads, half the SBUF/DMA.
  - Scores for a head PAIR run concurrently on the PE via row tiling:
    head 2p lives at partitions 0-63, head 2p+1 at 64-127 of dblk p, so
    the two K=64 matmuls land in distinct row-groups (tile_position
    (0,0)/(64,0) auto-derived) and overlap in the 128x128 array
    (measured issue delta: 3 ns).
  - qc-major schedule: attention for query chunk s interleaves the NEXT
    chunk's QKV projection matmuls and the PREVIOUS chunk's output
    projection as PE filler, so the scalar engine's exp hides under PE
    work. Extra filler is reserved for pair boundaries where the PV
    accumulator (PSUM) is released by the softmax-normalize chain.
  - PV lags scores by 2 groups (pend depth 2) so each block's exp has
    ~2 groups of PE work to complete before its PV needs it.

Device layouts (SBUF is [128 partitions, free]):
  x^T   [in=8*128, tok]      host-transposed activations
  Q^T/K^T [dout=4*128, tok]  head h occupies rows (h%2)*64.. of dblk h//2
  V     [tok, head, d_k+1]   65th column is ones so the PV matmul also
                             produces the softmax denominator row
  scores are computed transposed: S^T[k, q] = K @ Q^T, so softmax's sum
  over keys becomes a matmul contraction instead of a partition reduction.
"""

import os
import sys

for _p in (
    "/root/.axon_site",
    "/root/.axon_site/_ro/trn_rl_repo",
    "/root/.axon_site/_ro/pypackages",
    "/opt/trn_rl_repo",
):
    if os.path.isdir(_p) and _p not in sys.path:
        sys.path.append(_p)

import numpy as np

import concourse.mybir as mybir
import concourse.tile as tile
from concourse import bacc
from concourse.bass import ts
from concourse.bass_utils import run_bass_kernel_spmd

F32 = mybir.dt.float32
F16 = mybir.dt.float16
AF = mybir.ActivationFunctionType
ALU = mybir.AluOpType

B, S, HID = 4, 2048, 1024
HEADS, DK = 16, 64
NCORES = 8
HPC = HEADS // 2          # 8 heads per core
HSL = HPC * DK            # 512-dim hidden slice per core
TC = 512                  # token/query chunk
NTC = S // TC             # 4
NTB = S // 128            # 16 token blocks


def build_nc(debug_dumps=False):
    nc = bacc.Bacc("TRN2", target_bir_lowering=False, debug=False)

    # all inputs host-packed to the exact SBUF image [128, ...] so each DMA
    # moves long contiguous rows (1KB rows measured ~110 GB/s effective;
    # 8KB rows approach the 358 GB/s roofline)
    xT = nc.dram_tensor("xT", [128, NTC, 8, TC], F16, kind="ExternalInput").ap()
    wqT = nc.dram_tensor("wqT", [128, 8, HSL], F16, kind="ExternalInput").ap()
    wkT = nc.dram_tensor("wkT", [128, 8, HSL], F16, kind="ExternalInput").ap()
    wvT = nc.dram_tensor("wvT", [128, 8, HSL], F16, kind="ExternalInput").ap()
    woT = nc.dram_tensor("woT", [128, 4, HID], F16, kind="ExternalInput").ap()
    # biases pre-arranged host-side to [128, dblk] (a strided [512]->[128,4]
    # DMA costs ~4us in per-element descriptors)
    bq = nc.dram_tensor("bq", [128, 4], F32, kind="ExternalInput").ap()
    bk = nc.dram_tensor("bk", [128, 4], F32, kind="ExternalInput").ap()
    dmask = nc.dram_tensor("dmask", [128, 128], F16, kind="ExternalInput").ap()
    out = nc.dram_tensor("out_p", [S, HID], F16, kind="ExternalOutput").ap()
    if debug_dumps:
        qT_d = nc.dram_tensor("qT_d", [HSL, S], F16, kind="ExternalOutput").ap()
        kT_d = nc.dram_tensor("kT_d", [HSL, S], F16, kind="ExternalOutput").ap()
        v_d = nc.dram_tensor(
            "v_d", [128, NTB * HPC * (DK + 1)], F16, kind="ExternalOutput"
        ).ap()
        aT_d = nc.dram_tensor("aT_d", [HSL, S], F16, kind="ExternalOutput").ap()

    out_r = out.rearrange("(tb p) o -> p tb o", p=128)    # [128, 16, HID]

    with tile.TileContext(nc) as tc:
        with (
            tc.tile_pool(name="per", bufs=1) as per,
            tc.tile_pool(name="pt", bufs=4) as pt_pool,
            tc.tile_pool(name="sm", bufs=8) as sm_pool,
            tc.tile_pool(name="ot", bufs=2) as ot_pool,
            tc.tile_pool(name="psq", bufs=2, space="PSUM") as psq,
            tc.tile_pool(name="pss", bufs=2, space="PSUM") as pss,
            tc.tile_pool(name="pso", bufs=2, space="PSUM") as pso,
        ):
            # persistent SBUF state
            qT_sb = per.tile([128, 4, S], F16, tag="qT")
            kT_sb = per.tile([128, 4, S], F16, tag="kT")
            aT_sb = per.tile([128, 4, S], F16, tag="aT")
            v_sb = per.tile([128, NTB, HPC, DK + 1], F16, tag="v")
            x_sb = per.tile([128, NTC, 8, TC], F16, tag="x")

            wq_sb = per.tile([128, 8, HSL], F16, tag="wq")
            wk_sb = per.tile([128, 8, HSL], F16, tag="wk")
            wv_sb = per.tile([128, 8, HSL], F16, tag="wv")
            wo_sb = per.tile([128, 4, HID], F16, tag="wo")
            bq_sb = per.tile([128, 4], F32, tag="bq")
            bk_sb = per.tile([128, 4], F32, tag="bk")
            dm_sb = per.tile([128, 128], F16, tag="dm")

            # startup DMAs: the first-matmul gate is x0 + wq, so ONLY those
            # (plus the tiny bq/dm) are issued now, one per HWDGE queue —
            # issuing everything at once makes the gate contend for the
            # ~360GB/s aggregate. The rest are issued mid-prologue below.
            nc.sync.dma_start(x_sb[:, 0], xT[:, 0])
            nc.scalar.dma_start(wq_sb[:], wqT)
            nc.scalar.dma_start(bq_sb[:], bq)
            nc.scalar.dma_start(dm_sb[:], dmask)

            nc.vector.memset(v_sb[:, :, :, DK], 1.0)
            # preload the exp table set while DMAs run
            warm = sm_pool.tile([1, 8], F32, tag="warm")
            nc.vector.memset(warm[:], 0.0)
            nc.scalar.activation(warm[:], warm[:], AF.Exp, scale=1.0)

            # ---------------- QKV projection work items ----------------
            def qkv_items(tci):
                for w_sb, b_sb, dst in (
                    (wq_sb, bq_sb, qT_sb),
                    (wk_sb, bk_sb, kT_sb),
                ):
                    for dblk in range(4):
                        ps = psq.tile([128, TC], F32, tag="ps")
                        for ic in range(8):
                            yield (
                                lambda ps=ps, w=w_sb, d=dblk, ic=ic, tci=tci:
                                nc.tensor.matmul(
                                    ps[:],
                                    w[:, ic, ts(d, 128)],
                                    x_sb[:, tci, ic, :],
                                    start=(ic == 0),
                                    stop=(ic == 7),
                                )
                            )
                        yield (
                            lambda ps=ps, b=b_sb, d=dblk, dst=dst, tci=tci:
                            nc.vector.tensor_tensor(
                                dst[:, d, ts(tci, TC)],
                                ps[:],
                                b[:, d : d + 1].to_broadcast((128, TC)),
                                ALU.add,
                            )
                        )
                for tbl in range(4):
                    ps = psq.tile([128, TC], F32, tag="ps")
                    for ic in range(8):
                        yield (
                            lambda ps=ps, t=tbl, ic=ic, tci=tci:
                            nc.tensor.matmul(
                                ps[:],
                                x_sb[:, tci, ic, ts(t, 128)],
                                wv_sb[:, ic, :],
                                start=(ic == 0),
                                stop=(ic == 7),
                            )
                        )
                    tb = tci * 4 + tbl
                    # bv is dropped on-device: softmax weights sum to 1, so
                    # att = PV + bv and the host folds wo@bv into bo.
                    yield (
                        lambda ps=ps, tb=tb: nc.vector.tensor_copy(
                            v_sb[:, tb, :, 0:DK],
                            ps.rearrange("p (h d) -> p h d", d=DK),
                        )
                    )

            # ---------------- output projection work items ----------------
            def out_items(s):
                for tb in range(4 * s, 4 * s + 4):
                    ot = ot_pool.tile([128, HID], F16, tag="ot")
                    for half in range(2):
                        ps = psq.tile([128, TC], F32, tag="ps")
                        for hb in range(4):
                            yield (
                                lambda ps=ps, hb=hb, tb=tb, half=half:
                                nc.tensor.matmul(
                                    ps[:],
                                    aT_sb[:, hb, ts(tb, 128)],
                                    wo_sb[:, hb, ts(half, TC)],
                                    start=(hb == 0),
                                    stop=(hb == 3),
                                )
                            )
                        yield (
                            lambda ps=ps, ot=ot, half=half:
                            nc.vector.tensor_copy(ot[:, ts(half, TC)], ps[:])
                        )
                    yield lambda ot=ot, tb=tb: nc.sync.dma_start(
                        out_r[:, tb, :], ot[:]
                    )

            # ---------------- prologue: QKV for tci=0 ----------------
            pro = list(qkv_items(0))
            for it in pro[:9]:     # q/dblk0 unit: needs only wq+bq+x0
                it()
            nc.scalar.dma_start(wk_sb[:], wkT)
            nc.scalar.dma_start(bk_sb[:], bk)
            nc.sync.dma_start(x_sb[:, 1], xT[:, 1])
            for it in pro[9:36]:   # q/dblk1-3
                it()
            nc.scalar.dma_start(wv_sb[:], wvT)
            nc.sync.dma_start(x_sb[:, 2], xT[:, 2])
            for it in pro[36:72]:  # k units
                it()
            nc.scalar.dma_start(wo_sb[:], woT)
            nc.sync.dma_start(x_sb[:, 3], xT[:, 3])
            for it in pro[72:]:    # v units
                it()

            # ---------------- main qc-major pipeline ----------------
            for s in range(NTC):
                # filler distribution is balanced against each step's exp
                # load on the scalar engine (step s's exp work grows as
                # ~s+1): out-projections are deferred toward step 3, which
                # has no next-QKV to interleave
                fillers = []
                if s + 1 < NTC:
                    fillers.extend(qkv_items(s + 1))
                if s == 2:
                    fillers.extend(out_items(0))
                elif s == 3:
                    fillers.extend(out_items(1))
                    fillers.extend(out_items(2))
                fi = 0

                def pull(n):
                    nonlocal fi
                    end = min(fi + n, len(fillers))
                    while fi < end:
                        fillers[fi]()
                        fi += 1

                nkb = 4 * s + 4
                total_groups = 4 * nkb
                rpb = 12 if s < 3 else 8   # boundary reserve per pair
                spread = max(0, len(fillers) - 4 * rpb)
                g_done = 0
                pulled_sched = 0

                for p in range(4):
                    ops_e = pso.tile([DK + 1, TC], F32, tag="ops")
                    ops_o = pso.tile([DK + 1, TC], F32, tag="ops")
                    opsx = (ops_e, ops_o)

                    def emit_pv(pt, kb, cs, opsx=opsx, nkb=nkb, p=p):
                        for hi in range(2):
                            nc.tensor.matmul(
                                opsx[hi][:, cs:TC],
                                v_sb[:, kb, 2 * p + hi, :],
                                pt[:, hi, cs:TC],
                                start=(kb == 0),
                                stop=(kb == nkb - 1),
                            )

                    pend = []
                    for kb in range(nkb):
                        cs = max(0, kb * 128 - s * TC)
                        sp = pss.tile([128, 2, TC], F32, tag="sp")
                        for hi in range(2):
                            off = hi * 64
                            nc.tensor.matmul(
                                sp[:, hi, cs:TC],
                                kT_sb[off : off + 64, p, ts(kb, 128)],
                                qT_sb[off : off + 64, p, s * TC + cs : (s + 1) * TC],
                                start=True,
                                stop=True,
                            )
                        pt = pt_pool.tile([128, 2, TC], F16, tag="pt")
                        nc.scalar.activation(
                            pt[:, :, cs:TC], sp[:, :, cs:TC], AF.Exp, scale=0.125
                        )
                        if kb >= 4 * s:  # diagonal: zero the upper triangle
                            for hi in range(2):
                                nc.vector.tensor_tensor(
                                    pt[:, hi, cs : cs + 128],
                                    pt[:, hi, cs : cs + 128],
                                    dm_sb[:],
                                    ALU.mult,
                                )
                        if len(pend) == 2:
                            emit_pv(*pend.pop(0))
                        # evenly-paced filler to keep the PE dense
                        g_done += 1
                        want = spread * g_done // total_groups
                        pull(want - pulled_sched)
                        pulled_sched = want
                        pend.append((pt, kb, cs))
                    for args in pend:
                        emit_pv(*args)

                    # softmax normalize: aT[d, q] = ops[d, q] / ops[64, q].
                    # Phased so the two heads' chains overlap across DVE and
                    # gpsimd: copy both denoms, broadcast them RAW (gpsimd
                    # runs while DVE continues), reciprocal on the broadcast
                    # (DVE cost is free-size only — same price as [1, TC]),
                    # then multiply. Odd head first: its aT lands via DMA
                    # (partition shift), feeding the out-projection.
                    lsbs, bcss, rcps = {}, {}, {}
                    for hi in (1, 0):
                        lsb = sm_pool.tile([1, TC], F32, tag="lsb")
                        nc.vector.tensor_copy(lsb[:], opsx[hi][DK : DK + 1, :])
                        lsbs[hi] = lsb
                    for hi in (1, 0):
                        bcs = sm_pool.tile([DK, TC], F32, tag="bcs")
                        nc.gpsimd.partition_broadcast(bcs[:], lsbs[hi][:])
                        bcss[hi] = bcs
                    for hi in (1, 0):
                        rcp = sm_pool.tile([DK, TC], F32, tag="rcp")
                        nc.vector.reciprocal_approx_fast(rcp[:], bcss[hi][:])
                        rcps[hi] = rcp
                    for hi in (1, 0):
                        if hi == 0:
                            nc.vector.tensor_tensor(
                                aT_sb[0:DK, p, ts(s, TC)],
                                opsx[hi][0:DK, :],
                                rcps[hi][:],
                                ALU.mult,
                            )
                        else:
                            # engines are lane-locked; DMA shifts partitions
                            tmp = sm_pool.tile([DK, TC], F16, tag="tmp")
                            nc.vector.tensor_tensor(
                                tmp[:], opsx[hi][0:DK, :], rcps[hi][:], ALU.mult
                            )
                            nc.sync.dma_start(
                                aT_sb[DK:128, p, ts(s, TC)], tmp[:]
                            )
                    # boundary filler: covers the PSUM-release latency of the
                    # normalize chain before the next pair's first PV
                    pull(rpb)
                pull(len(fillers))

            for it in out_items(NTC - 1):
                it()

            if debug_dumps:
                nc.sync.dma_start(
                    qT_d.rearrange("(d p) t -> p d t", p=128), qT_sb[:]
                )
                nc.sync.dma_start(
                    kT_d.rearrange("(d p) t -> p d t", p=128), kT_sb[:]
                )
                nc.sync.dma_start(
                    v_d[:], v_sb.rearrange("p a b c -> p (a b c)")
                )
                nc.sync.dma_start(
                    aT_d.rearrange("(d p) t -> p d t", p=128), aT_sb[:]
                )
    nc.compile()
    return nc


_NC = None


def _get_nc():
    global _NC
    if _NC is None:
        _NC = build_nc()
    return _NC


def _numpy_reference(x, attn_mask, wq, bq, wk, bk, wv, bv, wo, bo):
    """Fallback for a non-causal mask (never hit with the standard inputs)."""
    Bsz, Seq, D = x.shape
    scale = 1.0 / np.sqrt(DK)

    def proj(w, b):
        y = x @ w.T + b
        return y.reshape(Bsz, Seq, HEADS, DK).transpose(0, 2, 1, 3)

    q, k, v = proj(wq, bq), proj(wk, bk), proj(wv, bv)
    scores = np.einsum("bhqd,bhkd->bhqk", q, k) * scale
    scores = np.where(attn_mask == 0, np.float32(-1e9), scores)
    scores = scores - scores.max(axis=-1, keepdims=True)
    p = np.exp(scores)
    p /= p.sum(axis=-1, keepdims=True)
    o = np.einsum("bhqk,bhkd->bhqd", p, v)
    o = o.transpose(0, 2, 1, 3).reshape(Bsz, Seq, D)
    return o @ wo.T + bo


def kernel(x, attn_mask, wq, bq, wk, bk, wv, bv, wo, bo, **_unused):
    x = np.asarray(x, np.float32)
    attn_mask = np.asarray(attn_mask)
    wq, bq = np.asarray(wq, np.float32), np.asarray(bq, np.float32)
    wk, bk = np.asarray(wk, np.float32), np.asarray(bk, np.float32)
    wv, bv = np.asarray(wv, np.float32), np.asarray(bv, np.float32)
    wo, bo = np.asarray(wo, np.float32), np.asarray(bo, np.float32)

    causal = np.array_equal(
        np.asarray(attn_mask).reshape(S, S) != 0, np.tril(np.ones((S, S), bool))
    )
    if not causal:
        return _numpy_reference(x, attn_mask, wq, bq, wk, bk, wv, bv, wo, bo)

    tri01 = (
        np.arange(128)[:, None] <= np.arange(128)[None, :]
    ).astype(np.float16)

    def pack(a, groups):
        # [(g p), cols] -> [p, g, cols]: the SBUF image, so DMA rows are
        # long and contiguous
        return np.ascontiguousarray(
            a.reshape(groups, 128, a.shape[1]).transpose(1, 0, 2)
        ).astype(np.float16)

    in_maps = []
    for c in range(NCORES):
        b, hg = c // 2, c % 2
        sl = slice(hg * HSL, (hg + 1) * HSL)
        # x packed tci-major: [p, tci, ic, tok] so each chunk DMA moves
        # 8KB contiguous rows
        xp = (
            x[b].T.astype(np.float16)
            .reshape(8, 128, NTC, TC)
            .transpose(1, 2, 0, 3)
        )
        in_maps.append(
            {
                "xT": np.ascontiguousarray(xp),
                "wqT": pack(wq[sl, :].T, 8),
                "wkT": pack(wk[sl, :].T, 8),
                "wvT": pack(wv[sl, :].T, 8),
                "woT": pack(wo[:, sl].T, 4),
                "bq": np.ascontiguousarray(bq[sl].reshape(4, 128).T),
                "bk": np.ascontiguousarray(bk[sl].reshape(4, 128).T),
                "dmask": tri01,
            }
        )

    res = run_bass_kernel_spmd(
        _get_nc(), in_maps, core_ids=list(range(NCORES)), **_RUN_KWARGS
    )
    if _RUN_RESULTS is not None:
        _RUN_RESULTS.append(res)

    # v bias is dropped on-device: softmax rows sum to 1, so att = PV + bv
    # exactly, and its projection wo @ bv folds into the output bias.
    bo2 = bo + wo @ bv
    out = np.empty((B, S, HID), np.float32)
    for b in range(B):
        out[b] = (
            res.results[2 * b]["out_p"].astype(np.float32)
            + res.results[2 * b + 1]["out_p"].astype(np.float32)
            + bo2
        )
    return out


# test.py can set these to enable tracing / inspect profile results.
_RUN_KWARGS = {}
_RUN_RESULTS = None


# revision 34
# speedup vs baseline: 1.0310x; 1.0310x over previous
"""Multi-head self-attention (B=4, S=2048, hidden=1024, 16 heads, d_k=64,
causal) on 8 Trainium2 NeuronCores.

Sharding: core c handles batch b = c//2 and head-group hg = c%2 (8 heads =
512 hidden dims). Each core computes Q/K/V for its heads, causal attention,
and a partial output projection against its wo column-slice; the host sums
the two partials per batch and adds the (bo + wo@bv) bias.

v3 design (fp16, pair-tiled, software-pipelined):
  - All matmul operands are fp16: 1 cycle/row on the PE (vs the fp32
    HIGH-power mode's ~2-3), FWL fast weight loads, half the SBUF/DMA.
  - Scores for a head PAIR run concurrently on the PE via row tiling:
    head 2p lives at partitions 0-63, head 2p+1 at 64-127 of dblk p, so
    the two K=64 matmuls land in distinct row-groups (tile_position
    (0,0)/(64,0) auto-derived) and overlap in the 128x128 array
    (measured issue delta: 3 ns).
  - qc-major schedule: attention for query chunk s interleaves the NEXT
    chunk's QKV projection matmuls and the PREVIOUS chunk's output
    projection as PE filler, so the scalar engine's exp hides under PE
    work. Extra filler is reserved for pair boundaries where the PV
    accumulator (PSUM) is released by the softmax-normalize chain.
  - PV lags scores by 2 groups (pend depth 2) so each block's exp has
    ~2 groups of PE work to complete before its PV needs it.

Device layouts (SBUF is [128 partitions, free]):
  x^T   [in=8*128, tok]      host-transposed activations
  Q^T/K^T [dout=4*128, tok]  head h occupies rows (h%2)*64.. of dblk h//2
  V     [tok, head, d_k+1]   65th column is ones so the PV matmul also
                             produces the softmax denominator row
  scores are computed transposed: S^T[k, q] = K @ Q^T, so softmax's sum
  over keys becomes a matmul contraction instead of a partition reduction.
"""

import os
import sys

for _p in (
    "/root/.axon_site",
    "/root/.axon_site/_ro/trn_rl_repo",
    "/root/.axon_site/_ro/pypackages",
    "/opt/trn_rl_repo",
):
    if os.path.isdir(_p) and _p not in sys.path:
        sys.path.append(_p)

import numpy as np

import concourse.mybir as mybir
import concourse.tile as tile
from concourse import bacc
from concourse.bass import ts
from concourse.bass_utils import run_bass_kernel_spmd

F32 = mybir.dt.float32
F16 = mybir.dt.float16
AF = mybir.ActivationFunctionType
ALU = mybir.AluOpType

B, S, HID = 4, 2048, 1024
HEADS, DK = 16, 64
NCORES = 8
HPC = HEADS // 2          # 8 heads per core
HSL = HPC * DK            # 512-dim hidden slice per core
TC = 512                  # token/query chunk
NTC = S // TC             # 4
NTB = S // 128            # 16 token blocks


def build_nc(debug_dumps=False):
    nc = bacc.Bacc("TRN2", target_bir_lowering=False, debug=False)

    # all inputs host-packed to the exact SBUF image [128, ...] so each DMA
    # moves long contiguous rows (1KB rows measured ~110 GB/s effective;
    # 8KB rows approach the 358 GB/s roofline)
    xT = nc.dram_tensor("xT", [128, NTC, 8, TC], F16, kind="ExternalInput").ap()
    wqT = nc.dram_tensor("wqT", [128, 8, HSL], F16, kind="ExternalInput").ap()
    wkT = nc.dram_tensor("wkT", [128, 8, HSL], F16, kind="ExternalInput").ap()
    wvT = nc.dram_tensor("wvT", [128, 8, HSL], F16, kind="ExternalInput").ap()
    woT = nc.dram_tensor("woT", [128, 4, HID], F16, kind="ExternalInput").ap()
    # biases pre-arranged host-side to [128, dblk] (a strided [512]->[128,4]
    # DMA costs ~4us in per-element descriptors)
    bq = nc.dram_tensor("bq", [128, 4], F32, kind="ExternalInput").ap()
    bk = nc.dram_tensor("bk", [128, 4], F32, kind="ExternalInput").ap()
    dmask = nc.dram_tensor("dmask", [128, 128], F16, kind="ExternalInput").ap()
    out = nc.dram_tensor("out_p", [S, HID], F16, kind="ExternalOutput").ap()
    if debug_dumps:
        qT_d = nc.dram_tensor("qT_d", [HSL, S], F16, kind="ExternalOutput").ap()
        kT_d = nc.dram_tensor("kT_d", [HSL, S], F16, kind="ExternalOutput").ap()
        v_d = nc.dram_tensor(
            "v_d", [128, NTB * HPC * (DK + 1)], F16, kind="ExternalOutput"
        ).ap()
        aT_d = nc.dram_tensor("aT_d", [HSL, S], F16, kind="ExternalOutput").ap()

    out_r = out.rearrange("(tb p) o -> p tb o", p=128)    # [128, 16, HID]

    with tile.TileContext(nc) as tc:
        with (
            tc.tile_pool(name="per", bufs=1) as per,
            tc.tile_pool(name="pt", bufs=4) as pt_pool,
            tc.tile_pool(name="sm", bufs=8) as sm_pool,
            tc.tile_pool(name="ot", bufs=2) as ot_pool,
            tc.tile_pool(name="psq", bufs=2, space="PSUM") as psq,
            tc.tile_pool(name="pss", bufs=2, space="PSUM") as pss,
            tc.tile_pool(name="pso", bufs=2, space="PSUM") as pso,
        ):
            # persistent SBUF state
            qT_sb = per.tile([128, 4, S], F16, tag="qT")
            kT_sb = per.tile([128, 4, S], F16, tag="kT")
            aT_sb = per.tile([128, 4, S], F16, tag="aT")
            v_sb = per.tile([128, NTB, HPC, DK + 1], F16, tag="v")
            x_sb = per.tile([128, NTC, 8, TC], F16, tag="x")

            wq_sb = per.tile([128, 8, HSL], F16, tag="wq")
            wk_sb = per.tile([128, 8, HSL], F16, tag="wk")
            wv_sb = per.tile([128, 8, HSL], F16, tag="wv")
            wo_sb = per.tile([128, 4, HID], F16, tag="wo")
            bq_sb = per.tile([128, 4], F32, tag="bq")
            bk_sb = per.tile([128, 4], F32, tag="bk")
            dm_sb = per.tile([128, 128], F16, tag="dm")

            # startup DMAs: the first-matmul gate is x0 + wq. The sync-queue
            # ring starts ~3µs before the scalar one (whose first slots also
            # sit behind the exp table load), so the gating transfers go on
            # sync and everything else streams on scalar.
            nc.sync.dma_start(x_sb[:, 0], xT[:, 0])
            nc.sync.dma_start(wq_sb[:], wqT)
            nc.sync.dma_start(bq_sb[:], bq)
            nc.scalar.dma_start(dm_sb[:], dmask)
            nc.scalar.dma_start(wk_sb[:], wkT)
            nc.scalar.dma_start(bk_sb[:], bk)
            nc.scalar.dma_start(wv_sb[:], wvT)
            nc.scalar.dma_start(wo_sb[:], woT)

            nc.vector.memset(v_sb[:, :, :, DK], 1.0)
            # preload the exp table set (after the DMA issues so the table
            # load doesn't delay the scalar queue's ring)
            warm = sm_pool.tile([1, 8], F32, tag="warm")
            nc.vector.memset(warm[:], 0.0)
            nc.scalar.activation(warm[:], warm[:], AF.Exp, scale=1.0)

            # ---------------- QKV projection work items ----------------
            def qkv_items(tci):
                for w_sb, b_sb, dst in (
                    (wq_sb, bq_sb, qT_sb),
                    (wk_sb, bk_sb, kT_sb),
                ):
                    for dblk in range(4):
                        ps = psq.tile([128, TC], F32, tag="ps")
                        for ic in range(8):
                            yield (
                                lambda ps=ps, w=w_sb, d=dblk, ic=ic, tci=tci:
                                nc.tensor.matmul(
                                    ps[:],
                                    w[:, ic, ts(d, 128)],
                                    x_sb[:, tci, ic, :],
                                    start=(ic == 0),
                                    stop=(ic == 7),
                                )
                            )
                        yield (
                            lambda ps=ps, b=b_sb, d=dblk, dst=dst, tci=tci:
                            nc.vector.tensor_tensor(
                                dst[:, d, ts(tci, TC)],
                                ps[:],
                                b[:, d : d + 1].to_broadcast((128, TC)),
                                ALU.add,
                            )
                        )
                for tbl in range(4):
                    ps = psq.tile([128, TC], F32, tag="ps")
                    for ic in range(8):
                        yield (
                            lambda ps=ps, t=tbl, ic=ic, tci=tci:
                            nc.tensor.matmul(
                                ps[:],
                                x_sb[:, tci, ic, ts(t, 128)],
                                wv_sb[:, ic, :],
                                start=(ic == 0),
                                stop=(ic == 7),
                            )
                        )
                    tb = tci * 4 + tbl
                    # bv is dropped on-device: softmax weights sum to 1, so
                    # att = PV + bv and the host folds wo@bv into bo.
                    yield (
                        lambda ps=ps, tb=tb: nc.vector.tensor_copy(
                            v_sb[:, tb, :, 0:DK],
                            ps.rearrange("p (h d) -> p h d", d=DK),
                        )
                    )

            # ---------------- output projection work items ----------------
            def out_items(s):
                for tb in range(4 * s, 4 * s + 4):
                    ot = ot_pool.tile([128, HID], F16, tag="ot")
                    for half in range(2):
                        ps = psq.tile([128, TC], F32, tag="ps")
                        for hb in range(4):
                            yield (
                                lambda ps=ps, hb=hb, tb=tb, half=half:
                                nc.tensor.matmul(
                                    ps[:],
                                    aT_sb[:, hb, ts(tb, 128)],
                                    wo_sb[:, hb, ts(half, TC)],
                                    start=(hb == 0),
                                    stop=(hb == 3),
                                )
                            )
                        yield (
                            lambda ps=ps, ot=ot, half=half:
                            nc.vector.tensor_copy(ot[:, ts(half, TC)], ps[:])
                        )
                    yield lambda ot=ot, tb=tb: nc.sync.dma_start(
                        out_r[:, tb, :], ot[:]
                    )

            # ---------------- prologue: QKV for tci=0 ----------------
            pro = list(qkv_items(0))
            for it in pro[:9]:     # q/dblk0 unit: needs only wq+bq+x0
                it()
            nc.sync.dma_start(x_sb[:, 1], xT[:, 1])
            for it in pro[9:36]:   # q/dblk1-3
                it()
            nc.sync.dma_start(x_sb[:, 2], xT[:, 2])
            for it in pro[36:72]:  # k units
                it()
            nc.sync.dma_start(x_sb[:, 3], xT[:, 3])
            for it in pro[72:]:    # v units
                it()

            # ---------------- main qc-major pipeline ----------------
            for s in range(NTC):
                # filler distribution is balanced against each step's exp
                # load on the scalar engine (step s's exp work grows as
                # ~s+1): out-projections are deferred toward step 3, which
                # has no next-QKV to interleave
                fillers = []
                if s + 1 < NTC:
                    fillers.extend(qkv_items(s + 1))
                if s == 2:
                    fillers.extend(out_items(0))
                elif s == 3:
                    fillers.extend(out_items(1))
                    fillers.extend(out_items(2))
                fi = 0

                def pull(n):
                    nonlocal fi
                    end = min(fi + n, len(fillers))
                    while fi < end:
                        fillers[fi]()
                        fi += 1

                nkb = 4 * s + 4
                total_groups = 4 * nkb
                rpb = 12 if s < 3 else 8   # boundary reserve per pair
                spread = max(0, len(fillers) - 4 * rpb)
                g_done = 0
                pulled_sched = 0

                for p in range(4):
                    ops_e = pso.tile([DK + 1, TC], F32, tag="ops")
                    ops_o = pso.tile([DK + 1, TC], F32, tag="ops")
                    opsx = (ops_e, ops_o)

                    def emit_pv(pt, kb, cs, opsx=opsx, nkb=nkb, p=p):
                        for hi in range(2):
                            nc.tensor.matmul(
                                opsx[hi][:, cs:TC],
                                v_sb[:, kb, 2 * p + hi, :],
                                pt[:, hi, cs:TC],
                                start=(kb == 0),
                                stop=(kb == nkb - 1),
                            )

                    pend = []
                    for kb in range(nkb):
                        cs = max(0, kb * 128 - s * TC)
                        sp = pss.tile([128, 2, TC], F32, tag="sp")
                        for hi in range(2):
                            off = hi * 64
                            nc.tensor.matmul(
                                sp[:, hi, cs:TC],
                                kT_sb[off : off + 64, p, ts(kb, 128)],
                                qT_sb[off : off + 64, p, s * TC + cs : (s + 1) * TC],
                                start=True,
                                stop=True,
                            )
                        pt = pt_pool.tile([128, 2, TC], F16, tag="pt")
                        nc.scalar.activation(
                            pt[:, :, cs:TC], sp[:, :, cs:TC], AF.Exp, scale=0.125
                        )
                        if kb >= 4 * s:  # diagonal: zero the upper triangle
                            for hi in range(2):
                                nc.vector.tensor_tensor(
                                    pt[:, hi, cs : cs + 128],
                                    pt[:, hi, cs : cs + 128],
                                    dm_sb[:],
                                    ALU.mult,
                                )
                        if len(pend) == 2:
                            emit_pv(*pend.pop(0))
                        # evenly-paced filler to keep the PE dense
                        g_done += 1
                        want = spread * g_done // total_groups
                        pull(want - pulled_sched)
                        pulled_sched = want
                        pend.append((pt, kb, cs))
                    for args in pend:
                        emit_pv(*args)

                    # softmax normalize: aT[d, q] = ops[d, q] / ops[64, q].
                    # Phased so the two heads' chains overlap across DVE and
                    # gpsimd: copy both denoms, broadcast them RAW (gpsimd
                    # runs while DVE continues), reciprocal on the broadcast
                    # (DVE cost is free-size only — same price as [1, TC]),
                    # then multiply. Odd head first: its aT lands via DMA
                    # (partition shift), feeding the out-projection.
                    lsbs, bcss, rcps = {}, {}, {}
                    for hi in (1, 0):
                        lsb = sm_pool.tile([1, TC], F32, tag="lsb")
                        nc.vector.tensor_copy(lsb[:], opsx[hi][DK : DK + 1, :])
                        lsbs[hi] = lsb
                    for hi in (1, 0):
                        bcs = sm_pool.tile([DK, TC], F32, tag="bcs")
                        nc.gpsimd.partition_broadcast(bcs[:], lsbs[hi][:])
                        bcss[hi] = bcs
                    # boundary filler HERE: its DVE work runs while gpsimd
                    # broadcasts, so the reciprocals below don't head-of-line
                    # block the DVE queue waiting on gpsimd
                    pull(rpb)
                    for hi in (1, 0):
                        rcp = sm_pool.tile([DK, TC], F32, tag="rcp")
                        nc.vector.reciprocal_approx_fast(rcp[:], bcss[hi][:])
                        rcps[hi] = rcp
                    for hi in (1, 0):
                        if hi == 0:
                            nc.vector.tensor_tensor(
                                aT_sb[0:DK, p, ts(s, TC)],
                                opsx[hi][0:DK, :],
                                rcps[hi][:],
                                ALU.mult,
                            )
                        else:
                            # engines are lane-locked; DMA shifts partitions
                            tmp = sm_pool.tile([DK, TC], F16, tag="tmp")
                            nc.vector.tensor_tensor(
                                tmp[:], opsx[hi][0:DK, :], rcps[hi][:], ALU.mult
                            )
                            # scalar queue: the sync queue carries 0.5MB
                            # output-DMA bursts that would delay this small
                            # staging transfer on the out-proj critical path
                            nc.scalar.dma_start(
                                aT_sb[DK:128, p, ts(s, TC)], tmp[:]
                            )
                pull(len(fillers))

            for it in out_items(NTC - 1):
                it()

            if debug_dumps:
                nc.sync.dma_start(
                    qT_d.rearrange("(d p) t -> p d t", p=128), qT_sb[:]
                )
                nc.sync.dma_start(
                    kT_d.rearrange("(d p) t -> p d t", p=128), kT_sb[:]
                )
                nc.sync.dma_start(
                    v_d[:], v_sb.rearrange("p a b c -> p (a b c)")
                )
                nc.sync.dma_start(
                    aT_d.rearrange("(d p) t -> p d t", p=128), aT_sb[:]
                )
    nc.compile()
    return nc


_NC = None


def _get_nc():
    global _NC
    if _NC is None:
        _NC = build_nc()
    return _NC


def _numpy_reference(x, attn_mask, wq, bq, wk, bk, wv, bv, wo, bo):
    """Fallback for a non-causal mask (never hit with the standard inputs)."""
    Bsz, Seq, D = x.shape
    scale = 1.0 / np.sqrt(DK)

    def proj(w, b):
        y = x @ w.T + b
        return y.reshape(Bsz, Seq, HEADS, DK).transpose(0, 2, 1, 3)

    q, k, v = proj(wq, bq), proj(wk, bk), proj(wv, bv)
    scores = np.einsum("bhqd,bhkd->bhqk", q, k) * scale
    scores = np.where(attn_mask == 0, np.float32(-1e9), scores)
    scores = scores - scores.max(axis=-1, keepdims=True)
    p = np.exp(scores)
    p /= p.sum(axis=-1, keepdims=True)
    o = np.einsum("bhqk,bhkd->bhqd", p, v)
    o = o.transpose(0, 2, 1, 3).reshape(Bsz, Seq, D)
    return o @ wo.T + bo


def kernel(x, attn_mask, wq, bq, wk, bk, wv, bv, wo, bo, **_unused):
    x = np.asarray(x, np.float32)
    attn_mask = np.asarray(attn_mask)
    wq, bq = np.asarray(wq, np.float32), np.asarray(bq, np.float32)
    wk, bk = np.asarray(wk, np.float32), np.asarray(bk, np.float32)
    wv, bv = np.asarray(wv, np.float32), np.asarray(bv, np.float32)
    wo, bo = np.asarray(wo, np.float32), np.asarray(bo, np.float32)

    causal = np.array_equal(
        np.asarray(attn_mask).reshape(S, S) != 0, np.tril(np.ones((S, S), bool))
    )
    if not causal:
        return _numpy_reference(x, attn_mask, wq, bq, wk, bk, wv, bv, wo, bo)

    tri01 = (
        np.arange(128)[:, None] <= np.arange(128)[None, :]
    ).astype(np.float16)

    def pack(a, groups):
        # [(g p), cols] -> [p, g, cols]: the SBUF image, so DMA rows are
        # long and contiguous
        return np.ascontiguousarray(
            a.reshape(groups, 128, a.shape[1]).transpose(1, 0, 2)
        ).astype(np.float16)

    in_maps = []
    for c in range(NCORES):
        b, hg = c // 2, c % 2
        sl = slice(hg * HSL, (hg + 1) * HSL)
        # x packed tci-major: [p, tci, ic, tok] so each chunk DMA moves
        # 8KB contiguous rows
        xp = (
            x[b].T.astype(np.float16)
            .reshape(8, 128, NTC, TC)
            .transpose(1, 2, 0, 3)
        )
        in_maps.append(
            {
                "xT": np.ascontiguousarray(xp),
                "wqT": pack(wq[sl, :].T, 8),
                "wkT": pack(wk[sl, :].T, 8),
                "wvT": pack(wv[sl, :].T, 8),
                "woT": pack(wo[:, sl].T, 4),
                "bq": np.ascontiguousarray(bq[sl].reshape(4, 128).T),
                "bk": np.ascontiguousarray(bk[sl].reshape(4, 128).T),
                "dmask": tri01,
            }
        )

    res = run_bass_kernel_spmd(
        _get_nc(), in_maps, core_ids=list(range(NCORES)), **_RUN_KWARGS
    )
    if _RUN_RESULTS is not None:
        _RUN_RESULTS.append(res)

    # v bias is dropped on-device: softmax rows sum to 1, so att = PV + bv
    # exactly, and its projection wo @ bv folds into the output bias.
    bo2 = bo + wo @ bv
    out = np.empty((B, S, HID), np.float32)
    for b in range(B):
        out[b] = (
            res.results[2 * b]["out_p"].astype(np.float32)
            + res.results[2 * b + 1]["out_p"].astype(np.float32)
            + bo2
        )
    return out


# test.py can set these to enable tracing / inspect profile results.
_RUN_KWARGS = {}
_RUN_RESULTS = None
